# revision 6
# baseline (speedup 1.0000x reference)
"""Trainium2 Bass kernel for nn_BertSelfOutput (BiT 8-bit quantized BertSelfOutput).

Computation (see reference):
    wq = sym_quant(weight, clip=2.5, bits=8)       # layerwise scale s_w = 127/max|clip(w)|
    xq = sym_quant(hidden_states, clip=2.5, bits=8)
    h  = xq @ wq.T + bias
    y  = LayerNorm(h + input_tensor) * gamma + beta

Sharding: data-parallel over batch (8 cores, 1 batch element each); weight/bias/LN
params replicated.  Host-side marshalling is pure relayout (transpose/reshape): x,
res and the weight are laid out so every DMA is contiguous per SBUF partition
(8KB+ descriptors, near line rate), with the contraction dim on partitions.

Device algorithm per core (streaming, DMA/PE co-limited):
  - s_x is a compile-time constant 127/2.5: the layerwise clip at 2.5 makes
    max|clip(x)| == 2.5 whenever any element of the 2M-sample N(0,1) shard clips,
    which is a certainty at this size (kernel() enforces it with an exact host-side
    prescale fallback for the impossible case).  This removes the global abs-max
    barrier, so x streams: DMA block -> ACT scale+round to i16 -> DVE clamp to
    integer-valued bf16 (or fp8) -> matmul, tile by tile.
  - s_w is computed on device: per-chunk DVE abs-max rides each w DMA piece,
    gpsimd partition_all_reduce folds partitions, reciprocal.  Quantization rounds
    via the f32->i16 convert (nearest-even, matches jnp.round); the +-127
    tensor_scalar clamp realizes the clip exactly.
  - integer matmul on the PE; fp32 PSUM accumulation is exact (|sum| < 2^24).
    The bias rides in as a K=1 *bf16* matmul (bias*s_x*s_w in bf16: ~0.4% of a term
    that is ~2% of y's rms -- negligible), so accumulation groups never see fp32.
  - USE_FP8: stores the quantized integers as fp8e4m3 and runs DoubleRow matmuls
    (2 MACs/cell/cycle, K=256 per instruction).  Integers above 16 round to 4
    significant bits, adding ~2-3% rms noise to the integer products -- measured
    ~1e-2 max rel err on the final output vs the 2e-2 gate.
  - LayerNorm is scale-invariant, so PSUM integers are never dequantized: the
    residual is scaled by s_x*s_w inside the fused scalar_tensor_tensor epilogue
    (which also emits the row sum), a GpSimd y*y scalar_tensor_tensor gives the
    sum of squares, batched stats -> rstd, ACT Identity applies (y-mu)*rstd.
  - output is stored bf16 tile-blocked (LN output is O(1); bf16 rounding is ~1e-3
    abs), widened to f32 and unblocked on the host (exact relayout).
"""

import numpy as np

P = 128
T = 2048   # tokens per core (S of one batch element)
H = 1024   # hidden
KO = 8     # k chunks of 128 (H / P)
XBLK = 2   # t-tiles per x/res/out DMA block
GROUP = 4  # t-tiles per stats group

S_X = 127.0 / 2.5  # layerwise activation quant scale (see module docstring)

USE_FP8 = False

_CACHE = {}


def _build(trivial_affine: bool, use_fp8: bool, t=T, h=H):
    import concourse.bass as bass
    import concourse.bass_isa as bass_isa
    import concourse.bacc as bacc
    import concourse.mybir as mybir
    import concourse.tile as tile

    ko = h // P
    nt = t // P                  # t-tiles
    nb = nt // XBLK             # x/res/out DMA blocks
    tb = XBLK * P               # tokens per block
    group = min(GROUP, nt)
    f32 = mybir.dt.float32
    bf16 = mybir.dt.bfloat16
    i16 = mybir.dt.int16
    qdt = mybir.dt.float8e4 if use_fp8 else bf16
    Alu = mybir.AluOpType
    Act = mybir.ActivationFunctionType

    nc = bacc.Bacc("TRN2", target_bir_lowering=False, debug=False)

    # x: [nb, P, ko*tb] f32, tile-contiguous: x4[b, p, c*tb + i] = x.T[c*P+p, b*tb + i]
    x4 = nc.dram_tensor("x4", [nb, P, ko * tb], f32, kind="ExternalInput").ap()
    # res: [nb, P, tb/P*h] f32 tile-blocked: res_m[b, p, q*h + o] = res[b*tb + q*P + p, o]
    res = nc.dram_tensor("res", [nb, P, XBLK * h], f32, kind="ExternalInput").ap()
    # w: [P, ko*h] f32: w3[p, c*h + o] = weight[o, c*P+p]
    w3 = nc.dram_tensor("w3", [P, ko * h], f32, kind="ExternalInput").ap()
    bias_d = nc.dram_tensor("bias", [h], f32, kind="ExternalInput").ap()
    gamma_d = nc.dram_tensor("gamma", [h], f32, kind="ExternalInput").ap()
    beta_d = nc.dram_tensor("beta", [h], f32, kind="ExternalInput").ap()
    # out: [nb, P, tb/P*h] bf16 tile-blocked (host unblocks + widens)
    out_d = nc.dram_tensor("out", [nb, P, XBLK * h], bf16, kind="ExternalOutput").ap()

    with tile.TileContext(nc) as tc:
        keep = tc.alloc_tile_pool(name="keep", bufs=1)
        p1 = tc.alloc_tile_pool(name="p1", bufs=1)

        # ---- persistent tiles ----
        wq = keep.tile([P, ko * h], qdt)   # quantized weight.T (integers)
        ones_bf = keep.tile([1, P], bf16)
        nc.vector.memset(ones_bf, 1.0)
        bias_sb = keep.tile([1, h], f32)
        nc.sync.dma_start(out=bias_sb, in_=bias_d[None, :])
        bias_bf = keep.tile([1, h], bf16)  # bias * s_x * s_w
        wmax8 = keep.tile([P, ko], f32)
        wmax_p = keep.tile([P, 1], f32)
        wmax_a = keep.tile([P, 1], f32)    # all-reduced |w| max (same value on all partitions)
        s_w = keep.tile([P, 1], f32)
        ssw = keep.tile([P, 1], f32)       # s_x * s_w (residual/bias pre-scale)
        stat_sum = keep.tile([P, nt], f32)
        stat_sq = keep.tile([P, nt], f32)
        mu = keep.tile([P, nt], f32)
        rstd = keep.tile([P, nt], f32)
        nmurs = keep.tile([P, nt], f32)    # -mu * rstd
        if not trivial_affine:
            gam_rep = keep.tile([P, h], f32)
            bet_rep = keep.tile([P, h], f32)
            nc.sync.dma_start(out=gam_rep, in_=gamma_d[None, :].to_broadcast((P, h)))
            nc.sync.dma_start(out=bet_rep, in_=beta_d[None, :].to_broadcast((P, h)))

        # ---- load weight (first: it gates the PE pipeline; FIFO ring gives it
        # priority over the x/res stream issued after it) ----
        wf = p1.tile([P, ko * h], f32)
        for c2 in range(4):
            sl = slice(c2 * 2 * h, (c2 + 1) * 2 * h)
            nc.sync.dma_start(out=wf[:, sl], in_=w3[:, sl])
        for c in range(ko):
            nc.vector.tensor_reduce(
                out=wmax8[:, c : c + 1], in_=wf[:, c * h : (c + 1) * h],
                axis=mybir.AxisListType.X, op=Alu.max, apply_absolute_value=True,
            )
        nc.vector.tensor_reduce(
            out=wmax_p, in_=wmax8, axis=mybir.AxisListType.X, op=Alu.max,
        )
        nc.gpsimd.partition_all_reduce(
            wmax_a, wmax_p, channels=P, reduce_op=bass_isa.ReduceOp.absmax,
        )
        # m = min(max|w|, clip); the +-127 clamp after rounding realizes the clip
        nc.vector.tensor_scalar_min(out=wmax_a, in0=wmax_a, scalar1=2.5)
        nc.vector.reciprocal(out=s_w, in_=wmax_a)
        nc.vector.tensor_scalar_mul(out=s_w, in0=s_w, scalar1=127.0)
        nc.vector.tensor_scalar_mul(out=ssw, in0=s_w, scalar1=S_X)
        nc.vector.tensor_scalar_mul(out=bias_bf, in0=bias_sb, scalar1=ssw[0:1, 0:1])

        # quantize weight: round(w*s_w) clamp [-127,127].  The HW f32->i16 convert
        # rounds nearest-even (matches jnp.round); min/max apply the clip during
        # the i16 -> bf16/fp8 convert (the clamped integers are bf16-exact).
        for c in range(ko):
            wi16 = p1.tile([P, h], i16, tag="wi16", name=f"wi16_{c}", bufs=2)
            nc.scalar.activation(
                out=wi16, in_=wf[:, c * h : (c + 1) * h], func=Act.Identity,
                scale=s_w, bias=0.0,
            )
            nc.vector.tensor_scalar(
                out=wq[:, c * h : (c + 1) * h], in0=wi16, scalar1=127.0,
                scalar2=-127.0, op0=Alu.min, op1=Alu.max,
            )

        # ---- streaming main loop ----
        pool_xf = tc.alloc_tile_pool(name="xf", bufs=2)
        pool_xi = tc.alloc_tile_pool(name="xi", bufs=2)
        pool_xq = tc.alloc_tile_pool(name="xq", bufs=3)
        pool_rt = tc.alloc_tile_pool(name="rt", bufs=3)
        pool_yt = tc.alloc_tile_pool(name="yt", bufs=2 * group)
        pool_sq = tc.alloc_tile_pool(name="sq", bufs=2)
        pool_ot = tc.alloc_tile_pool(name="ot", bufs=2)
        pool_ps = tc.alloc_tile_pool(name="ps", bufs=3, space="PSUM")

        half = h // 2
        yts = {}
        for b in range(nb):
            xf = pool_xf.tile([P, ko * tb], f32, tag="xf", name=f"xf_{b}")
            nc.sync.dma_start(out=xf, in_=x4[b])
            xi = pool_xi.tile([P, ko * tb], i16, tag="xi", name=f"xi_{b}")
            nc.scalar.activation(out=xi, in_=xf, func=Act.Identity, scale=S_X, bias=0.0)
            xq = pool_xq.tile([P, ko * tb], qdt, tag="xq", name=f"xq_{b}")
            nc.vector.tensor_scalar(
                out=xq, in0=xi, scalar1=127.0, scalar2=-127.0, op0=Alu.min, op1=Alu.max,
            )
            rt = pool_rt.tile([P, XBLK * h], f32, tag="rt", name=f"rt_{b}")
            nc.sync.dma_start(out=rt, in_=res[b])
            if use_fp8:
                xq_v = xq.rearrange("p (c k t) -> p c k t", c=ko // 2, k=2)
                wq_v = wq.rearrange("p (c k o) -> p c k o", c=ko // 2, k=2)

            for q in range(XBLK):
                j = XBLK * b + q
                ps = pool_ps.tile([P, h], f32, tag="ps", name=f"ps_{j}")
                for nf in range(2):
                    ocol = slice(nf * half, (nf + 1) * half)
                    nc.tensor.matmul(
                        ps[:, ocol], lhsT=ones_bf, rhs=bias_bf[:, ocol],
                        start=True, stop=False,
                    )
                if use_fp8:
                    for c in range(ko // 2):
                        lhs = xq_v[:, c, :, q * P : (q + 1) * P]
                        for nf in range(2):
                            ocol = slice(nf * half, (nf + 1) * half)
                            nc.tensor.matmul(
                                ps[:, ocol], lhsT=lhs, rhs=wq_v[:, c, :, ocol],
                                start=False, stop=(c == ko // 2 - 1),
                                perf_mode=mybir.MatmulPerfMode.DoubleRow,
                            )
                else:
                    for c in range(ko):
                        lhs = xq[:, c * tb + q * P : c * tb + (q + 1) * P]
                        for nf in range(2):
                            ocol = slice(nf * half + c * h, (nf + 1) * half + c * h)
                            nc.tensor.matmul(
                                ps[:, ocol.start - c * h : ocol.stop - c * h],
                                lhsT=lhs, rhs=wq[:, ocol],
                                start=False, stop=(c == ko - 1),
                            )
                # y = res*(s_x*s_w) + psum ; accum_out = row-sum of y
                yt = pool_yt.tile([P, h], f32, tag="yt", name=f"yt_{j}")
                yts[j] = yt
                nc.vector.scalar_tensor_tensor(
                    out=yt, in0=rt[:, q * h : (q + 1) * h], scalar=ssw, in1=ps,
                    op0=Alu.mult, op1=Alu.add,
                    accum_out=stat_sum[:, j : j + 1],
                )
                # sum of squares on ACT (output tensor is a throwaway)
                sq = pool_sq.tile([P, h], bf16, tag="sq", name=f"sq_{j}")
                nc.scalar.activation(
                    out=sq, in_=yt, func=Act.Square,
                    accum_out=stat_sq[:, j : j + 1],
                )

            if (b + 1) * XBLK % group == 0:
                # ---- batched stats for the group ----
                g0 = (b + 1) * XBLK - group
                gsl = slice(g0, g0 + group)
                musl = mu[:, gsl]
                nc.vector.tensor_scalar_mul(out=musl, in0=stat_sum[:, gsl], scalar1=1.0 / h)
                var = rstd[:, gsl]  # slot reused: var -> sd -> rstd
                nc.vector.tensor_scalar_mul(out=var, in0=stat_sq[:, gsl], scalar1=1.0 / h)
                mu2 = pool_sq.tile([P, group], f32, tag="mu2", name=f"mu2_{g0}")
                nc.vector.tensor_tensor(mu2, musl, musl, Alu.mult)
                nc.vector.tensor_tensor(var, var, mu2, Alu.subtract)
                nc.scalar.sqrt(out=var, in_=var)
                nc.vector.reciprocal(out=var, in_=var)
                nc.vector.tensor_tensor(nmurs[:, gsl], musl, var, Alu.mult)
                nc.vector.tensor_scalar_mul(out=nmurs[:, gsl], in0=nmurs[:, gsl], scalar1=-1.0)
                # ---- normalize + store (per DMA block; stores ride the scalar
                # HWDGE ring so they never stall the x/res load ring) ----
                for b2 in range(g0 // XBLK, (g0 + group) // XBLK):
                    ot = pool_ot.tile([P, XBLK * h], bf16, tag="ot", name=f"ot_{b2}")
                    for q in range(XBLK):
                        j = XBLK * b2 + q
                        yt = yts.pop(j)
                        osl = slice(q * h, (q + 1) * h)
                        # (y - mu) * rstd on DVE (2x single-tensor mode), bf16 out
                        nc.vector.tensor_scalar(
                            out=ot[:, osl], in0=yt,
                            scalar1=rstd[:, j : j + 1], scalar2=nmurs[:, j : j + 1],
                            op0=Alu.mult, op1=Alu.add,
                        )
                        if not trivial_affine:
                            nc.vector.tensor_tensor(ot[:, osl], ot[:, osl], gam_rep, Alu.mult)
                            nc.vector.tensor_tensor(ot[:, osl], ot[:, osl], bet_rep, Alu.add)
                    nc.scalar.dma_start(out=out_d[b2], in_=ot)

        for p in (pool_ps, pool_ot, pool_sq, pool_yt, pool_rt, pool_xq, pool_xi, pool_xf, p1, keep):
            p.release()

    if not nc.is_finalized():
        nc.finalize()
    return nc


def _get_nc(trivial_affine: bool, t=T, h=H):
    key = (trivial_affine, USE_FP8, t, h)
    if key not in _CACHE:
        _CACHE[key] = _build(trivial_affine, USE_FP8, t, h)
    return _CACHE[key]


def make_in_maps(hidden_states, input_tensor, weight, bias, gamma, beta):
    """Host-side marshalling: pure relayout except the (practically impossible)
    no-clip fallback, where an exact prescale keeps the device math identical to
    the reference (see module docstring)."""
    hidden_states = np.asarray(hidden_states, dtype=np.float32)
    input_tensor = np.asarray(input_tensor, dtype=np.float32)
    weight = np.asarray(weight, dtype=np.float32)
    bias = np.asarray(bias, dtype=np.float32)
    gamma = np.asarray(gamma, dtype=np.float32)
    beta = np.asarray(beta, dtype=np.float32)

    B, S, HH = hidden_states.shape
    ko = HH // P
    nb = S // P // XBLK
    tb = XBLK * P

    m = float(np.abs(hidden_states).max())
    if m < 2.5:
        # reference scale would be 127/m; prescaling x/res/bias by 2.5/m makes
        # round(x'*S_X) the exact reference integers and S_X*res' the exact
        # reference residual scaling (LN is scale-invariant).
        f = 2.5 / m
        hidden_states = hidden_states * f
        input_tensor = input_tensor * f
        bias = bias * f

    w3 = np.ascontiguousarray(
        weight.T.reshape(ko, P, HH).transpose(1, 0, 2)
    ).reshape(P, ko * HH)
    in_maps = []
    for c in range(B):
        x4 = np.ascontiguousarray(
            hidden_states[c].reshape(nb, tb, ko, P).transpose(0, 3, 2, 1)
        ).reshape(nb, P, ko * tb)
        res_m = np.ascontiguousarray(
            input_tensor[c].reshape(nb, XBLK, P, HH).transpose(0, 2, 1, 3)
        ).reshape(nb, P, XBLK * HH)
        in_maps.append(
            {
                "x4": x4,
                "res": res_m,
                "w3": w3,
                "bias": bias,
                "gamma": gamma,
                "beta": beta,
            }
        )
    return in_maps


def gather_out(results, B, S=T, HH=H):
    """Unblock the tiled bf16 output and widen to f32 (exact relayout)."""
    nb = S // P // XBLK
    outs = []
    for c in range(B):
        o = np.asarray(results[c]["out"]).astype(np.float32)
        outs.append(o.reshape(nb, P, XBLK, HH).transpose(0, 2, 1, 3).reshape(S, HH))
    return np.stack(outs)


def kernel(hidden_states, input_tensor, weight, bias, gamma, beta):
    from concourse.bass_utils import run_bass_kernel_spmd

    gamma = np.asarray(gamma, dtype=np.float32)
    beta = np.asarray(beta, dtype=np.float32)
    B, S, HH = np.asarray(hidden_states).shape
    trivial = bool(np.all(gamma == 1.0) and np.all(beta == 0.0))
    nc = _get_nc(trivial, S, HH)

    in_maps = make_in_maps(hidden_states, input_tensor, weight, bias, gamma, beta)
    r = run_bass_kernel_spmd(nc, in_maps, core_ids=list(range(B)))
    return gather_out(r.results, B, S, HH)
